# revision 5
# baseline (speedup 1.0000x reference)
"""CG coupler (segment_reduce) Trainium2 kernel.

out[b, ro[t]] += x1[b, r1[t]] * x2[b, r2[t]] * cg[t]

Structure detected from the runtime index tables: T splits into 147 runs of
128 consecutive 128-aligned indices with a constant coefficient per run,
giving 147 slot-level terms out[:,so] += c * x1[:,s1] (*) x2[:,s2] over 70
distinct (s1, s2) slot products and 16 output slots.

Per-core schedule (data-parallel over batch, 512 rows/core, fp16 on-chip):
 - rows processed in 4 chunks of 128 (partition dim = row), double-buffered
 - DVE computes the 70 pair products per chunk in ~13 batched tensor_tensor
   ops: the lex-sorted pair list decomposes into unit-stride runs, so one op
   covers a whole run with the fixed operand stride-0 broadcast
 - (so,|c|)-duplicate term groups are pre-added (add/sub chains) on DVE and
   GPSIMD into extra product slots, each merge removing one matmul
 - PE scatter-accumulates every term into PSUM via c-scaled-identity fp16
   matmuls (1 cycle/row); accumulation groups are kept sequential per PSUM
   bank (HW requirement) while interleaving across banks by product
   availability; warm-up filler MMs hold the PE p-state high
 - ACT evacuates each PSUM bank (f32 -> fp16) as soon as its matmuls finish
   (per-bank PSUM tiles keep the dependency tracking fine-grained)
 - DMA: block-0 inputs land as two feature pieces matched to the run order
   so products start ~3us earlier; the weight table is split by first use;
   per-chunk output rows leave in a banks-0-2 piece and a bank-3 piece
"""

import sys

if "/opt/trn_rl_repo" not in sys.path:
    sys.path.append("/opt/trn_rl_repo")

from contextlib import ExitStack

import numpy as np

import concourse.mybir as mybir
import concourse.tile as tile
from concourse import bacc
from concourse.bass_utils import run_bass_kernel_spmd

N_CORES = 8
P = 128
F16 = mybir.dt.float16
F32 = mybir.dt.float32
MULT = None  # set after import in _build (AluOpType)

_CACHE: dict = {}

# ---- tuning knobs -----------------------------------------------------------
import os as _os

N_FILLER = int(_os.environ.get("K_FILLER", "80"))   # PE warm-up MMs (N=64)
FILLER_N = 64          # free dim of each filler matmul
N_PRE = int(_os.environ.get("K_PRE", "16"))         # pre-add ops (1 MM saved each)
N_PRE_POOL = int(_os.environ.get("K_PRE_POOL", "10"))  # of those, on GPSIMD
N_POOL_RUNS = int(_os.environ.get("K_POOL_RUNS", "0"))  # product runs on GPSIMD
POOL_TERM_TARGET = 0   # unused (STT terms on Pool are not HW-valid)


def _detect_terms(r1, r2, ro, cg, in_dim, out_dim):
    T = len(cg)
    if T % P != 0 or len(r1) != T or len(r2) != T or len(ro) != T:
        return None
    d1 = np.diff(r1)
    d2 = np.diff(r2)
    do = np.diff(ro)
    brk = np.where(~((d1 == 1) & (d2 == 1) & (do == 1)))[0] + 1
    starts = np.concatenate([[0], brk])
    ends = np.concatenate([brk, [T]])
    if not np.all(ends - starts == P):
        return None
    a0, b0, o0 = r1[starts], r2[starts], ro[starts]
    if (a0 % P).any() or (b0 % P).any() or (o0 % P).any():
        return None
    if a0.max() + P > in_dim or b0.max() + P > in_dim or o0.max() + P > out_dim:
        return None
    cg2 = np.asarray(cg).reshape(-1, P)
    if not np.all(cg2 == cg2[:, :1]):
        return None
    return list(
        zip(
            (a0 // P).tolist(),
            (b0 // P).tolist(),
            (o0 // P).tolist(),
            cg2[:, 0].astype(np.float64).tolist(),
        )
    )


def _numpy_fallback(x1, x2, cg, r1, r2, ro, out_dim):
    out = np.zeros((x1.shape[0], out_dim), dtype=x1.dtype)
    prod = x1[:, r1] * x2[:, r2] * cg[None, :].astype(x1.dtype)
    np.add.at(out, (slice(None), ro), prod)
    return out


def _plan(terms, n_so):
    """Host-side schedule construction. Returns a dict with:
    runs, pair_idx, pool_slots, pool_ops, pre_adds, pe_mms, coeffs."""
    pairs = sorted(set((s1, s2) for s1, s2, _, _ in terms))
    pair_idx = {p: i for i, p in enumerate(pairs)}

    # unit-stride runs over the lex-sorted pair list
    runs = []
    cur = [pairs[0]]
    for p in pairs[1:]:
        if len(cur) >= 2:
            d = (cur[1][0] - cur[0][0], cur[1][1] - cur[0][1])
            if (p[0] - cur[-1][0], p[1] - cur[-1][1]) == d:
                cur.append(p)
                continue
        elif len(cur) == 1:
            d = (p[0] - cur[0][0], p[1] - cur[0][1])
            if d in ((0, 1), (1, 0), (1, 1)):
                cur.append(p)
                continue
        runs.append(cur)
        cur = [p]
    runs.append(cur)
    # execution order: unlock runs with the smallest input-feature footprint
    # first so products can start before the full block-0 DMA lands
    def run_need(r):
        hi1 = max(s1 for s1, _ in r) + 1
        hi2 = max(s2 for _, s2 in r) + 1
        return max(hi1, hi2)

    runs.sort(key=lambda r: (run_need(r), len(r)))

    # feature split point for the half-tiles (see b0_pieces below)
    n_slots_in = max(max(s1 for s1, _ in pairs), max(s2 for _, s2 in pairs)) + 1
    early = max((run_need(r) for r in runs if run_need(r) < n_slots_in), default=0)
    if not (0 < early < n_slots_in):
        early = n_slots_in

    # split runs that cross the half-tile boundary on either coordinate so
    # every product op reads from exactly one half-tile per operand
    def split_runs(rr):
        out = []
        for r in rr:
            cur = [r[0]]
            for p in r[1:]:
                if (p[0] >= early) != (cur[0][0] >= early) or (
                    p[1] >= early
                ) != (cur[0][1] >= early):
                    out.append(cur)
                    cur = [p]
                else:
                    cur.append(p)
            out.append(cur)
        return out

    runs = split_runs(runs)

    # the last N_POOL_RUNS runs execute on GPSIMD concurrently with the DVE
    # stream (they only depend on the input DMAs)
    pool_runs = set(range(len(runs) - N_POOL_RUNS, len(runs)))

    # availability rank of each product = index of the run op producing it
    avail = {}
    for ri, r in enumerate(runs):
        for p in r:
            avail[pair_idx[p]] = 3.0 if ri in pool_runs else ri

    # block-0 DMA split: two feature pieces per tensor (matching the
    # half-tiles). The first piece covers every run with a sub-full
    # footprint so products start early.
    if early < n_slots_in:
        b0_pieces = [(1, 0, early), (2, 0, early), (1, early, n_slots_in),
                     (2, early, n_slots_in)]
    else:
        b0_pieces = [(1, 0, n_slots_in), (2, 0, n_slots_in)]

    # x1's A-half splits once more: the leading runs broadcast a single x1
    # slot each, so a small first DMA covering just those slots lets the
    # product stream start ~0.5us earlier
    x1a_split = None
    if early < n_slots_in:
        pre_s1 = []
        s2hi0 = max(p[1] for p in runs[0]) + 1
        for r in runs:
            s1set = {p[0] for p in r}
            if len(s1set) == 1 and max(p[1] for p in r) + 1 <= s2hi0:
                s1 = next(iter(s1set))
                if s1 < early:
                    pre_s1.append(s1)
                    continue
            break
        if pre_s1:
            lo, hi = min(pre_s1), max(pre_s1) + 1
            if sorted(set(pre_s1)) == list(range(lo, hi)) and 0 < lo < hi == early:
                x1a_split = (lo, hi)

    # per-slot term counts -> pick pool slots greedily from slots {0}+suffix
    cnt = [0] * n_so
    for _, _, so, _ in terms:
        cnt[so] += 1
    pool_slots: set = set()
    total = 0
    if POOL_TERM_TARGET > 0:
        for so in (0, n_so - 1, n_so - 2, n_so - 3, n_so - 4):
            if total + cnt[so] <= POOL_TERM_TARGET + 4 and total < POOL_TERM_TARGET:
                pool_slots.add(so)
                total += cnt[so]
    # PE slots must form one contiguous range for a single-op evacuation
    pe_slots = [so for so in range(n_so) if so not in pool_slots]
    assert pe_slots == list(range(min(pe_slots), max(pe_slots) + 1)), (
        pe_slots,
        pool_slots,
    )

    # groups by (so, c) among PE slots; pre-add the biggest groups on DVE
    # (a group of g products collapses into one MM via a g-1 op add chain
    # accumulated in-place into one extra product slot)
    from collections import defaultdict

    # group by (so, |c|): sign differences are absorbed by add/subtract ops
    groups = defaultdict(list)
    for s1, s2, so, c in terms:
        groups[(so, round(abs(c), 12))].append((pair_idx[(s1, s2)], c))
    pre_adds = []   # (dst, a, b, engine, trigger, subtract): dst may repeat
    pre_mms = []    # (so, c_ref, dst_prod_idx)
    consumed = set()
    n_prod = len(pairs)
    n_extra = 0
    # prefer groups whose inputs are available earliest (late preadds stall
    # the in-order PE stream at chunk boundaries), then larger groups
    def group_trig(plist):
        return max(avail[p] for p, _ in plist)

    ordered = sorted(
        groups.items(), key=lambda kv: (group_trig(kv[1]), -len(kv[1]))
    )
    # crude queue model: each engine drains preadds at a fraction of a run op
    q_avail = {"pool": 0.0, "dve": 0.0}
    q_cost = {"pool": 0.7, "dve": 0.3}
    for (so, _ca), plist in ordered:
        if len(pre_adds) >= N_PRE:
            break
        if so in pool_slots or len(plist) < 2:
            continue
        # whole chain on one engine (cross-engine RMW ping-pong is wasteful)
        eng = "pool" if len(pre_adds) < N_PRE_POOL else "dve"
        p0, c0 = plist[0]
        p1, c1 = plist[1]
        npi = n_prod + n_extra
        n_extra += 1
        trig = max(avail[p0], avail[p1])
        pre_adds.append((npi, p0, p1, eng, trig, c1 * c0 < 0))
        done = max(trig + 0.6, q_avail[eng] + q_cost[eng])
        for extra, ce in plist[2:]:
            trig = max(trig, avail[extra])
            pre_adds.append((npi, npi, extra, eng, trig, ce * c0 < 0))
            done = max(trig + 0.6, done + q_cost[eng])
        q_avail[eng] = done
        avail[npi] = done
        pre_mms.append((so, c0, npi))
        consumed.add((so, round(abs(c0), 12)))
    n_prod_tot = n_prod + n_extra

    # PE matmul list: (so, c, prod_idx) for every non-pool, non-merged term.
    # HW constraint (verified): accumulation groups within one PSUM bank must
    # not interleave -> per bank, emit each slot's MM group contiguously;
    # across banks, merge by product availability.
    pe_mms_all = list(pre_mms)
    for s1, s2, so, c in terms:
        if so in pool_slots or (so, round(abs(c), 12)) in consumed:
            continue
        pe_mms_all.append((so, c, pair_idx[(s1, s2)]))
    pe_lo = pe_slots[0]
    slot_mms = defaultdict(list)
    for m in pe_mms_all:
        slot_mms[m[0]].append(m)
    for so in slot_mms:
        slot_mms[so].sort(key=lambda m: avail[m[2]])
    bank_queues = defaultdict(list)
    for so in sorted(slot_mms, key=lambda so: max(avail[m[2]] for m in slot_mms[so])):
        bank_queues[(so - pe_lo) // 4].append(so)
    queues = [
        [m for so in slots for m in slot_mms[so]] for slots in bank_queues.values()
    ]
    pe_mms = []
    heads = [0] * len(queues)
    while any(h < len(q) for h, q in zip(heads, queues)):
        best = min(
            (i for i in range(len(queues)) if heads[i] < len(queues[i])),
            key=lambda i: avail[queues[i][heads[i]][2]],
        )
        pe_mms.append(queues[best][heads[best]])
        heads[best] += 1

    # pool op list: (so, c, prod_idx, is_first)
    pool_terms = defaultdict(list)
    for s1, s2, so, c in terms:
        if so in pool_slots:
            pool_terms[so].append((c, pair_idx[(s1, s2)]))
    pool_ops = []
    for so, tl in pool_terms.items():
        tl.sort(key=lambda t: avail[t[1]])
        for j, (c, pi) in enumerate(tl):
            pool_ops.append((so, c, pi, j == 0))
    pool_ops.sort(key=lambda o: (avail[o[2]], not o[3]))

    # weights ordered by first use so the weight table can be DMA'd in two
    # pieces with the early piece unblocking the first matmuls
    coeffs = []
    for _, c, _ in pe_mms:
        c = round(c, 12)
        if c not in coeffs:
            coeffs.append(c)
    return {
        "early": early,
        "pairs": pairs,
        "runs": runs,
        "pair_idx": pair_idx,
        "pool_slots": sorted(pool_slots),
        "pe_slots": pe_slots,
        "pre_adds": pre_adds,
        "pe_mms": pe_mms,
        "pool_ops": pool_ops,
        "coeffs": coeffs,
        "n_prod": n_prod,
        "n_prod_tot": n_prod_tot,
        "b0_pieces": b0_pieces,
        "pool_runs": pool_runs,
        "x1a_split": x1a_split,
    }


def _build_program(plan, b_shard, in_dim, out_dim):
    add = mybir.AluOpType.add
    mult = mybir.AluOpType.mult

    nblk = b_shard // P
    n_so = out_dim // P
    pairs = plan["pairs"]
    runs = plan["runs"]
    pair_idx = plan["pair_idx"]
    pe_slots = plan["pe_slots"]
    pre_adds = plan["pre_adds"]
    pe_mms = plan["pe_mms"]
    pool_ops = plan["pool_ops"]
    coeffs = plan["coeffs"]
    n_prod_tot = plan["n_prod_tot"]
    widx = {c: i for i, c in enumerate(coeffs)}
    pe_lo = pe_slots[0]
    n_pe = len(pe_slots)
    assert n_pe * P <= 2048

    # start/stop bookkeeping per (slot): first/last MM in issue order
    slot_mm_count = {}
    for so, _, _ in pe_mms:
        slot_mm_count[so] = slot_mm_count.get(so, 0) + 1

    nc = bacc.Bacc("TRN2", target_bir_lowering=False, debug=False)
    x1d = nc.dram_tensor("x1", [b_shard, in_dim], F16, kind="ExternalInput").ap()
    x2d = nc.dram_tensor("x2", [b_shard, in_dim], F16, kind="ExternalInput").ap()
    wtsd = nc.dram_tensor(
        "wts", [P, len(coeffs) * P], F16, kind="ExternalInput"
    ).ap()
    outd = nc.dram_tensor("out", [b_shard, out_dim], F16, kind="ExternalOutput").ap()

    early = plan["early"]
    n_in_slots = in_dim // P
    splitx = early < n_in_slots
    f_split = early * P  # feature column where the half-tiles split
    n_wt_a = min(8, len(coeffs))

    with tile.TileContext(nc) as tc, ExitStack() as ctx:
        sb = ctx.enter_context(tc.tile_pool(name="sb", bufs=1))
        ps = ctx.enter_context(tc.tile_pool(name="ps", bufs=1, space="PSUM"))

        # block 0 lands in split half-tiles (fine-grained DMA deps for the
        # pipeline head); blocks 1+ use whole tiles (fewer DMAs)
        loA = f_split
        loB = in_dim - f_split
        x1a_split = plan["x1a_split"]
        if x1a_split:
            s_lo = x1a_split[0] * P
            X1A1 = sb.tile([P, loA - s_lo], F16, tag="X1A1")
            X1A2 = sb.tile([P, s_lo], F16, tag="X1A2")
            X1A = None
        else:
            X1A = sb.tile([P, loA], F16, tag="X1A")
        X2A = sb.tile([P, loA], F16, tag="X2A")
        X1B = sb.tile([P, loB], F16, tag="X1B", name="X1B") if splitx else None
        X2B = sb.tile([P, loB], F16, tag="X2B", name="X2B") if splitx else None
        X1Rs = []
        X2Rs = []
        for b in range(1, nblk):
            t1 = sb.tile([P, in_dim], F16, tag=f"X1R{b}", name=f"X1R{b}")
            t2 = sb.tile([P, in_dim], F16, tag=f"X2R{b}", name=f"X2R{b}")
            X1Rs.append(t1)
            X2Rs.append(t2)
        PR0 = sb.tile([P, n_prod_tot * P], F16, tag="PR0")
        PR1 = sb.tile([P, n_prod_tot * P], F16, tag="PR1")
        ST0A = sb.tile([P, 1536], F16, tag="ST0A")
        ST0B = sb.tile([P, out_dim - 1536], F16, tag="ST0B")
        ST1_0 = sb.tile([P, 512], F16, tag="ST1_0")
        ST1_1 = sb.tile([P, 512], F16, tag="ST1_1")
        ST1_2 = sb.tile([P, 512], F16, tag="ST1_2")
        ST1_3 = sb.tile([P, 512], F16, tag="ST1_3")
        WTSA = sb.tile([P, n_wt_a * P], F16, tag="WTSA")
        WTSB = (
            sb.tile([P, (len(coeffs) - n_wt_a) * P], F16, tag="WTSB", name="WTSB")
            if len(coeffs) > n_wt_a
            else None
        )
        MEMS = sb.tile([P, P], F16, tag="MEMS")
        # one PSUM tile per bank so the per-bank evacuation only depends on
        # that bank's matmuls (dep tracking is tile-granular)
        PS00 = ps.tile([P, 512], F32, tag="PS00")
        PS01 = ps.tile([P, 512], F32, tag="PS01")
        PS02 = ps.tile([P, 512], F32, tag="PS02")
        PS03 = ps.tile([P, 512], F32, tag="PS03")
        PS10 = ps.tile([P, 512], F32, tag="PS10")
        PS11 = ps.tile([P, 512], F32, tag="PS11")
        PS12 = ps.tile([P, 512], F32, tag="PS12")
        PS13 = ps.tile([P, 512], F32, tag="PS13")
        prods = [PR0, PR1]
        stgs = [(ST0A, ST0B), (ST1_0, ST1_1, ST1_2, ST1_3)]
        psums = [[PS00, PS01, PS02, PS03], [PS10, PS11, PS12, PS13]]

        def wslice(wi):
            if wi < n_wt_a:
                return WTSA[:, wi * P : (wi + 1) * P]
            wi -= n_wt_a
            return WTSB[:, wi * P : (wi + 1) * P]

        def xslice(tid, b, slo, shi):
            """[P, (shi-slo)*P] view of x<tid> block b, slots [slo, shi)."""
            if b > 0:
                R = X1Rs[b - 1] if tid == 1 else X2Rs[b - 1]
                return R[:, slo * P : shi * P]
            if tid == 1 and x1a_split and (not splitx or shi <= early):
                lo1 = x1a_split[0]
                if shi <= lo1:
                    return X1A2[:, slo * P : shi * P]
                assert slo >= lo1, (slo, shi, x1a_split)
                return X1A1[:, (slo - lo1) * P : (shi - lo1) * P]
            A, Bt = (X1A, X1B) if tid == 1 else (X2A, X2B)
            if not splitx or shi <= early:
                return A[:, slo * P : shi * P]
            assert slo >= early
            return Bt[:, (slo - early) * P : (shi - early) * P]

        nc.gpsimd.memset(MEMS[:], 0.0)

        # input DMAs: block-0 halves first (products start on the A halves),
        # then the first weight piece, then the remaining blocks whole
        def load_b0_tile(X, xd, flo, fhi):
            nc.sync.dma_start(
                out=X[:].rearrange("p (one f) -> p one f", one=1),
                in_=xd[0:P, flo:fhi].rearrange("(one p) f -> p one f", p=P),
            )

        def load_b0(xd, half):
            if half == 0:
                X, flo, fhi = (X1A, 0, f_split) if xd is x1d else (X2A, 0, f_split)
            else:
                X, flo, fhi = (
                    (X1B, f_split, in_dim) if xd is x1d else (X2B, f_split, in_dim)
                )
            load_b0_tile(X, xd, flo, fhi)

        if x1a_split:
            load_b0_tile(X1A1, x1d, x1a_split[0] * P, f_split)
            load_b0(x2d, 0)
            load_b0_tile(X1A2, x1d, 0, x1a_split[0] * P)
        else:
            load_b0(x1d, 0)
            load_b0(x2d, 0)
        nc.sync.dma_start(out=WTSA[:], in_=wtsd[:, 0 : n_wt_a * P])
        if splitx:
            load_b0(x1d, 1)
            load_b0(x2d, 1)
        if WTSB is not None:
            nc.sync.dma_start(out=WTSB[:], in_=wtsd[:, n_wt_a * P :])
        for b in range(1, nblk):
            for xd, R in ((x1d, X1Rs[b - 1]), (x2d, X2Rs[b - 1])):
                nc.sync.dma_start(
                    out=R[:].rearrange("p (one f) -> p one f", one=1),
                    in_=xd[b * P : (b + 1) * P, :].rearrange(
                        "(one p) f -> p one f", p=P
                    ),
                )

        # PE warm-up fillers (overwritten by chunk 0's start=True term MMs)
        for i in range(N_FILLER):
            nc.tensor.matmul(
                out=psums[0][0][:, :FILLER_N],
                lhsT=MEMS[:],
                rhs=MEMS[:, :FILLER_N],
                start=True,
                stop=True,
            )

        for chunk in range(nblk):
            par = chunk % 2
            PR = prods[par]
            stg = stgs[par]
            PS = psums[par]
            xoff = chunk * in_dim

            # ---- DVE: batched products (preadds interleaved by readiness) --
            dve_pre = sorted(
                (pa for pa in pre_adds if pa[3] == "dve"), key=lambda p: p[4]
            )
            pool_pre = sorted(
                (pa for pa in pre_adds if pa[3] == "pool"), key=lambda p: p[4]
            )

            def emit_pre(builder, npi, a, b, sub):
                builder.tensor_tensor(
                    out=PR[:, npi * P : (npi + 1) * P],
                    in0=PR[:, a * P : (a + 1) * P],
                    in1=PR[:, b * P : (b + 1) * P],
                    op=mybir.AluOpType.subtract if sub else add,
                )

            def emit_run(builder, r):
                g = len(r)
                lo = pair_idx[r[0]]
                out_ap = PR[:, lo * P : (lo + g) * P].rearrange(
                    "p (g ch) -> p g ch", ch=P
                )
                (s1a, s2a) = r[0]
                d = (0, 1) if g == 1 else (r[1][0] - r[0][0], r[1][1] - r[0][1])
                if d == (0, 1):
                    in0 = (
                        xslice(1, chunk, s1a, s1a + 1)
                        .unsqueeze(1)
                        .broadcast_to((P, g, P))
                    )
                    in1 = xslice(2, chunk, s2a, s2a + g).rearrange(
                        "p (g ch) -> p g ch", ch=P
                    )
                elif d == (1, 0):
                    in0 = xslice(1, chunk, s1a, s1a + g).rearrange(
                        "p (g ch) -> p g ch", ch=P
                    )
                    in1 = (
                        xslice(2, chunk, s2a, s2a + 1)
                        .unsqueeze(1)
                        .broadcast_to((P, g, P))
                    )
                elif d == (1, 1):
                    in0 = xslice(1, chunk, s1a, s1a + g).rearrange(
                        "p (g ch) -> p g ch", ch=P
                    )
                    in1 = xslice(2, chunk, s2a, s2a + g).rearrange(
                        "p (g ch) -> p g ch", ch=P
                    )
                else:
                    raise AssertionError(d)
                builder.tensor_tensor(out=out_ap, in0=in0, in1=in1, op=mult)

            # GPSIMD-assigned runs first: they depend only on the input DMAs,
            # so the Pool engine computes them concurrently with the DVE runs
            for ri in sorted(plan["pool_runs"]):
                emit_run(nc.gpsimd, runs[ri])

            di = 0
            pi_ = 0
            for ri, r in enumerate(runs):
                if ri in plan["pool_runs"]:
                    continue
                emit_run(nc.vector, r)
                # program order defines tile dataflow deps: emit each preadd
                # only after the run that completes its inputs
                while di < len(dve_pre) and dve_pre[di][4] <= ri:
                    npi, a, b, _, _, sub = dve_pre[di]
                    emit_pre(nc.vector, npi, a, b, sub)
                    di += 1
                while pi_ < len(pool_pre) and pool_pre[pi_][4] <= ri:
                    npi, a, b, _, _, sub = pool_pre[pi_]
                    emit_pre(nc.gpsimd, npi, a, b, sub)
                    pi_ += 1
            for npi, a, b, _, _, sub in dve_pre[di:]:
                emit_pre(nc.vector, npi, a, b, sub)
            for npi, a, b, _, _, sub in pool_pre[pi_:]:
                emit_pre(nc.gpsimd, npi, a, b, sub)

            # ---- PE: term matmuls ----
            # last chunk: emit bank-by-bank so early banks finish first and
            # the per-bank evacuation/out-DMA below drains while PE still runs
            mm_seq = pe_mms
            if chunk == nblk - 1:
                mm_seq = sorted(pe_mms, key=lambda m: (m[0] - pe_lo) // 4)
            seen = {}
            for so, c, pi in mm_seq:
                seen[so] = seen.get(so, 0) + 1
                rel = so - pe_lo
                bk, off = divmod(rel * P, 512)
                nc.tensor.matmul(
                    out=PS[bk][:, off : off + P],
                    lhsT=wslice(widx[round(c, 12)]),
                    rhs=PR[:, pi * P : (pi + 1) * P],
                    start=(seen[so] == 1),
                    stop=(seen[so] == slot_mm_count[so]),
                )

            # ---- ACT: evacuate each PSUM bank as its matmuls finish.
            # Even parity: banks 0-2 stage together, bank 3 separately.
            # Odd parity (incl. the last chunk): per-bank staging so each
            # 512-col piece of the output row leaves as soon as its bank is
            # done -- this shortens the pipeline tail after the final MM ----
            n_banks = (n_pe * P + 511) // 512
            for bk in range(n_banks):
                w = min(512, n_pe * P - bk * 512)
                if par == 0:
                    if bk < 3:
                        dst = stg[0][:, bk * 512 : bk * 512 + w]
                    else:
                        dst = stg[1][:, (bk - 3) * 512 : (bk - 3) * 512 + w]
                else:
                    dst = stg[bk][:, 0:w]
                nc.scalar.copy(out=dst, in_=PS[bk][:, 0:w])
                if par == 1:
                    nc.sync.dma_start(
                        out=outd[
                            chunk * P : (chunk + 1) * P, bk * 512 : bk * 512 + w
                        ].rearrange("(one p) f -> p one f", p=P),
                        in_=stg[bk][:, 0:w].rearrange(
                            "p (one f) -> p one f", one=1
                        ),
                    )
            if par == 0:
                nc.sync.dma_start(
                    out=outd[chunk * P : (chunk + 1) * P, 0:1536].rearrange(
                        "(one p) f -> p one f", p=P
                    ),
                    in_=stg[0][:].rearrange("p (one f) -> p one f", one=1),
                )
                nc.sync.dma_start(
                    out=outd[chunk * P : (chunk + 1) * P, 1536:out_dim].rearrange(
                        "(one p) f -> p one f", p=P
                    ),
                    in_=stg[1][:].rearrange("p (one f) -> p one f", one=1),
                )

    nc.finalize()
    return nc


def _make_wts(coeffs):
    w = np.zeros((P, len(coeffs) * P), dtype=np.float16)
    for i, c in enumerate(coeffs):
        w[np.arange(P), i * P + np.arange(P)] = np.float16(c)
    return w


def kernel(x1, x2, cg_tilde, repids_in1, repids_in2, repids_out, out_dim):
    x1 = np.ascontiguousarray(np.asarray(x1, dtype=np.float32))
    x2 = np.ascontiguousarray(np.asarray(x2, dtype=np.float32))
    cg = np.asarray(cg_tilde, dtype=np.float32)
    r1 = np.asarray(repids_in1).astype(np.int64)
    r2 = np.asarray(repids_in2).astype(np.int64)
    ro = np.asarray(repids_out).astype(np.int64)
    out_dim = int(np.asarray(out_dim))

    B, in_dim = x1.shape
    terms = None
    if (
        B % (N_CORES * 4 * P) == 0
        and in_dim % P == 0
        and out_dim % P == 0
        and x2.shape == x1.shape
    ):
        terms = _detect_terms(r1, r2, ro, cg, in_dim, out_dim)
    if terms is None:
        return _numpy_fallback(x1, x2, cg, r1, r2, ro, out_dim)

    b_shard = B // N_CORES
    key = (B, in_dim, out_dim, np.asarray(terms, dtype=np.float64).tobytes())
    ent = _CACHE.get(key)
    if ent is None:
        plan = _plan(terms, out_dim // P)
        nc = _build_program(plan, b_shard, in_dim, out_dim)
        ent = (nc, plan)
        _CACHE[key] = ent
    nc, plan = ent

    wts = _make_wts(plan["coeffs"])
    x1h = x1.astype(np.float16)
    x2h = x2.astype(np.float16)
    in_maps = [
        {
            "x1": x1h[i * b_shard : (i + 1) * b_shard],
            "x2": x2h[i * b_shard : (i + 1) * b_shard],
            "wts": wts,
        }
        for i in range(N_CORES)
    ]
    res = run_bass_kernel_spmd(nc, in_maps, core_ids=list(range(N_CORES)))
    out = np.concatenate([res.results[i]["out"] for i in range(N_CORES)], axis=0)
    return out.astype(np.float32)
